# revision 25
# baseline (speedup 1.0000x reference)
"""Fused conv3x3 -> GroupNorm(16) -> channel scale -> maxpool2x2 -> clamp[0,1]
Trainium2 Bass kernel, data-parallel over batch on 8 NeuronCores.

Input  x [32, 64, 128, 128] f32  -> output [32, 128, 63, 63] f32.
Each core handles 4 samples.

Conv in fp16 (1 cycle/column on the PE vs 3 for fp32-HIGH; input rounding
error after GroupNorm is ~5e-3, well inside the 2e-2 gate). Tap-wise
matmuls: partition p = ci + 64*r holds x[ci, row+r, :] (r in {0,1}), so one
matmul with a [128, 128] stacked weight covers taps (kh=0, kw) and (kh=1,
kw) at once; kh=2 taps run as 64-partition matmuls off the low half.
6 matmuls per 4 output rows.

Post-processing pools BEFORE the GroupNorm affine: sign(gn_w*scale) is
folded into the conv weights on the host so the affine slope A is always
>= 0, making maxpool commute with it. Per 8-row PSUM chunk: ACT evacuates
to fp16, DVE takes bn_stats straight from PSUM, GPSIMD maxes row pairs;
once per sample GPSIMD maxes column pairs, ACT applies A*y+B with fused
Relu on the pooled 63x63 tile, GPSIMD clamps the top at 1.
"""

import numpy as np

import concourse.bacc as bacc
import concourse.mybir as mybir
import concourse.tile as tile

N_CORES = 8
B_FULL, CIN, H, W = 32, 64, 128, 128
COUT = 128
BPC = B_FULL // N_CORES  # samples per core
OH = OW = 126
PH = PW = 63
NG = 16  # groups
GSZ = COUT // NG  # 8 channels per group
EPS = 1e-5

# (out_row0, n_out_rows) conv chunks; x rows [r0 .. r0+nor+1] feed each
CHUNKS = [(16 * i, 16) for i in range(7)] + [(112, 14)]
NSTATS = 32  # bn_stats slots per sample

_CACHED = {}


def _build():
    if "nc" in _CACHED:
        return _CACHED["nc"]
    f32 = mybir.dt.float32
    f16 = mybir.dt.float16
    AF = mybir.ActivationFunctionType
    OP = mybir.AluOpType

    nc = bacc.Bacc("TRN2", target_bir_lowering=False, debug=False)
    xs = nc.dram_tensor("xs", [BPC, CIN, H, W], f16, kind="ExternalInput").ap()
    wp_d = nc.dram_tensor("wp", [3, 128, COUT], f16, kind="ExternalInput").ap()
    ws_d = nc.dram_tensor("ws", [3, 64, COUT], f16, kind="ExternalInput").ap()
    cb_d = nc.dram_tensor("cb", [COUT, 1], f32, kind="ExternalInput").ap()
    sg_d = nc.dram_tensor("sg", [COUT, 1], f32, kind="ExternalInput").ap()
    gws_d = nc.dram_tensor("gws", [COUT, 1], f32, kind="ExternalInput").ap()
    gsa_d = nc.dram_tensor("gsa", [COUT, 1], f32, kind="ExternalInput").ap()
    gbs_d = nc.dram_tensor("gbs", [COUT, 1], f32, kind="ExternalInput").ap()
    bones_d = nc.dram_tensor("bones", [COUT, COUT], f32, kind="ExternalInput").ap()
    out_d = nc.dram_tensor("out", [BPC, COUT, PH, PW], f16, kind="ExternalOutput").ap()

    with tile.TileContext(nc) as tc:
        with (
            tc.tile_pool(name="consts", bufs=1) as cpool,
            tc.tile_pool(name="xpool", bufs=3) as xpool,
            tc.tile_pool(name="ypool", bufs=4) as ypool,
            tc.tile_pool(name="vpool", bufs=2) as vpool,
            tc.tile_pool(name="hpool", bufs=2) as hpool,
            tc.tile_pool(name="zpool", bufs=2) as zpool,
            tc.tile_pool(name="stpool", bufs=2) as stpool,
            tc.tile_pool(name="cps", bufs=3, space="PSUM") as cps,
            tc.tile_pool(name="gps", bufs=1, space="PSUM") as gps,
        ):
            wp = cpool.tile([128, 3 * COUT], f16, name="wp_t")
            ws = cpool.tile([64, 3 * COUT], f16, name="ws_t")
            for kw in range(3):
                nc.sync.dma_start(wp[:, kw * COUT : (kw + 1) * COUT], wp_d[kw])
                nc.sync.dma_start(ws[:, kw * COUT : (kw + 1) * COUT], ws_d[kw])
            cb = cpool.tile([COUT, 1], f32, name="cb_t")
            nc.sync.dma_start(cb[:], cb_d[:])
            sg = cpool.tile([COUT, 1], f32, name="sg_t")
            nc.sync.dma_start(sg[:], sg_d[:])
            gws = cpool.tile([COUT, 1], f32, name="gws_t")
            nc.sync.dma_start(gws[:], gws_d[:])
            gsa = cpool.tile([COUT, 1], f32, name="gsa_t")
            nc.sync.dma_start(gsa[:], gsa_d[:])
            gbs = cpool.tile([COUT, 1], f32, name="gbs_t")
            nc.sync.dma_start(gbs[:], gbs_d[:])
            bones = cpool.tile([COUT, COUT], f32, name="bones_t")
            nc.sync.dma_start(bones[:], bones_d[:])
            zeros1 = cpool.tile([COUT, 1], f32, name="zeros1")
            nc.vector.memset(zeros1[:], 0.0)
            epst = cpool.tile([COUT, 1], f32, name="epst")
            nc.vector.memset(epst[:], EPS)

            # Warmup: keep the PE busy while the first x chunk loads so the
            # HAM clock-gate reaches 8/8 before the real stream starts.
            # Depends only on a memset, so it starts as soon as the PE is up.
            wmov = cpool.tile([128, 384], f16, name="wmov")
            nc.vector.memset(wmov[:], 0.0)
            wps = gps.tile([128, 384], f32, tag="warm", name="warm_ps")
            NWARM = 24
            for i in range(NWARM):
                nc.tensor.matmul(
                    wps[:], wmov[:, 0:COUT], wmov[:],
                    start=(i == 0), stop=(i == NWARM - 1),
                )
            wjunk = cpool.tile([128, 1], f32, name="wjunk")
            nc.vector.tensor_reduce(
                wjunk[:], wps[:], mybir.AxisListType.X, OP.max
            )

            def make_postlude(b, stats, vp):
                def postlude():
                    # per-channel -> per-group affine coefficients (mostly
                    # on the Scalar engine; [128,1] ops via bias/scale APs)
                    mv = stpool.tile([128, 2], f32, tag="mv", name="mv")
                    nc.vector.bn_aggr(mv[:], stats[:])
                    st = stpool.tile([128, 2], f32, tag="sts", name="st")
                    # t1 = sg*mean' + cb ; t2 = var' + t1^2 — all on the
                    # Vector engine right after bn_aggr (no cross-engine
                    # hops on the critical path to the aggregation matmul)
                    nc.vector.tensor_scalar(
                        st[:, 0:1], mv[:, 0:1], sg[:], cb[:], OP.mult, OP.add
                    )
                    t1sq = stpool.tile([128, 1], f32, tag="t1sq", name="t1sq")
                    nc.vector.tensor_tensor(
                        t1sq[:], st[:, 0:1], st[:, 0:1], OP.mult
                    )
                    nc.vector.tensor_tensor(
                        st[:, 1:2], mv[:, 1:2], t1sq[:], OP.add
                    )
                    gsum = gps.tile([128, 2], f32, tag="gsum", name="gsum")
                    nc.tensor.matmul(gsum[:], bones[:], st[:], start=True,
                                     stop=True)
                    mgrp = stpool.tile([128, 1], f32, tag="mgrp", name="mgrp")
                    nc.scalar.activation(
                        mgrp[:], gsum[:, 0:1], AF.Copy, scale=1.0 / GSZ
                    )
                    vgrp = stpool.tile([128, 1], f32, tag="vgrp", name="vgrp")
                    nc.scalar.activation(
                        vgrp[:], gsum[:, 1:2], AF.Identity, bias=epst[:],
                        scale=1.0 / GSZ
                    )
                    msq = stpool.tile([128, 1], f32, tag="msq", name="msq")
                    nc.scalar.activation(msq[:], mgrp[:], AF.Square)
                    nc.vector.tensor_tensor(vgrp[:], vgrp[:], msq[:],
                                            OP.subtract)
                    sdev = stpool.tile([128, 1], f32, tag="sdev", name="sdev")
                    nc.scalar.activation(sdev[:], vgrp[:], AF.Sqrt,
                                         bias=zeros1[:])
                    inv = stpool.tile([128, 1], f32, tag="inv", name="inv")
                    nc.vector.reciprocal(inv[:], sdev[:])
                    # Apos = inv*|gws| (>=0) ; B2 = inv*gws*(cb-mgrp) + gbs
                    apos = stpool.tile([128, 1], f32, tag="Ap", name="apos")
                    nc.scalar.activation(apos[:], gsa[:], AF.Copy,
                                         scale=inv[:])
                    sa = stpool.tile([128, 1], f32, tag="sA", name="sa")
                    nc.scalar.activation(sa[:], gws[:], AF.Copy, scale=inv[:])
                    b2 = stpool.tile([128, 1], f32, tag="B2", name="b2")
                    nc.vector.tensor_tensor(b2[:], cb[:], mgrp[:], OP.subtract)
                    nc.scalar.activation(b2[:], b2[:], AF.Identity,
                                         bias=gbs[:], scale=sa[:])

                    # column-pair max, then affine+clamp: first row-half on
                    # the Scalar engine, second on the Vector engine so the
                    # two halves finish in parallel
                    hp = hpool.tile([128, PH, PW], f16, tag="hp", name="hp")
                    zout = zpool.tile([128, PH * PW], f16, tag="z", name="zout")
                    halves = ((0, 32), (32, PH))
                    for r0, r1 in halves:
                        nc.vector.tensor_tensor(
                            hp[:, r0:r1, :],
                            vp[:, r0:r1, 0 : OW : 2],
                            vp[:, r0:r1, 1 : OW : 2],
                            OP.max
                        )
                    r0, r1 = halves[0]
                    seg0 = zout[:, r0 * PW : r1 * PW]
                    nc.scalar.activation(
                        seg0.rearrange("p (a b) -> p a b", b=PW),
                        hp[:, r0:r1, :], AF.Relu, bias=b2[:], scale=apos[:]
                    )
                    nc.vector.tensor_scalar(seg0, seg0, 1.0, None, OP.min)
                    r0, r1 = halves[1]
                    seg1 = zout[:, r0 * PW : r1 * PW]
                    nc.vector.tensor_scalar(
                        seg1.rearrange("p (a b) -> p a b", b=PW),
                        hp[:, r0:r1, :], apos[:], b2[:], OP.mult, OP.add
                    )
                    nc.vector.tensor_scalar(seg1, seg1, 1.0, 0.0, OP.min,
                                            OP.max)
                    for r0, r1 in halves:
                        nc.sync.dma_start(
                            out_d[b, :, r0:r1, :].rearrange("c h w -> c (h w)"),
                            zout[:, r0 * PW : r1 * PW],
                        )
                return postlude

            pending = None
            for b in range(BPC):
                stats = stpool.tile([128, NSTATS, 6], f32, tag="st", name="stats")
                vp = vpool.tile([128, PH, OW], f16, tag="vp", name="vp")
                si = 0
                for ci_, (c0, nor) in enumerate(CHUNKS):
                    xt = xpool.tile([128, 18, W], f16, tag="x", name="xt")
                    # low half: rows c0 .. c0+nor+1 (kh0 + source of kh2 taps);
                    # the very first load is split across dynamic DMA queues
                    # so the conv stream can start sooner
                    if b == 0 and ci_ == 0:
                        for s0, s1 in ((0, 6), (6, 12), (12, nor + 2)):
                            nc.gpsimd.dma_start(
                                xt[0:64, s0:s1, :], xs[b, :, c0 + s0 : c0 + s1, :]
                            )
                    else:
                        nc.gpsimd.dma_start(
                            xt[0:64, 0 : nor + 2, :],
                            xs[b, :, c0 : c0 + nor + 2, :]
                        )
                    # high half: rows c0+1 .. c0+nor (kh1 taps), copied
                    # SBUF->SBUF from the low half (saves HBM reads); the
                    # first chunk reads HBM directly so it doesn't serialize
                    # behind the low half
                    if b == 0 and ci_ == 0:
                        nc.gpsimd.dma_start(
                            xt[64:128, 0:nor, :],
                            xs[b, :, c0 + 1 : c0 + 1 + nor, :]
                        )
                    else:
                        nc.gpsimd.dma_start(
                            xt[64:128, 0:nor, :], xt[0:64, 1 : 1 + nor, :]
                        )

                    g0 = 0  # chunk-local output row
                    while g0 < nor:
                        gn = min(8, nor - g0)  # 8 or 6 (last chunk tail)
                        hr0, hr1 = (4, 4) if gn == 8 else (4, 2)
                        cp = cps.tile([128, 1024], f32, tag="cp", name="cp")
                        for hi, hr in ((0, hr0), (1, hr1)):
                            l0 = g0 + hi * 4
                            outap = cp[:, hi * 512 : hi * 512 + hr * OW]
                            for kw in range(3):
                                nc.tensor.matmul(
                                    outap,
                                    wp[:, kw * COUT : (kw + 1) * COUT],
                                    xt[:, l0 : l0 + hr, kw : kw + OW],
                                    start=(kw == 0),
                                    stop=False,
                                )
                            for kw in range(3):
                                nc.tensor.matmul(
                                    outap,
                                    ws[:, kw * COUT : (kw + 1) * COUT],
                                    xt[0:64, l0 + 2 : l0 + 2 + hr, kw : kw + OW],
                                    start=False,
                                    stop=(kw == 2),
                                )
                        # evacuate to fp16 (ACT), stats from PSUM (DVE),
                        # row-pair max (GPSIMD)
                        y16 = ypool.tile([128, 8 * OW], f16, tag="y", name="y16")
                        cpv = cp[:].rearrange("p (h w) -> p h w", w=512)
                        if gn == 8:
                            nc.scalar.activation(
                                y16[:].rearrange("p (h w) -> p h w", w=4 * OW),
                                cpv[:, :, 0 : 4 * OW],
                                AF.Copy,
                            )
                        else:
                            nc.scalar.activation(
                                y16[:, 0 : 4 * OW], cp[:, 0 : 4 * OW], AF.Copy
                            )
                            nc.scalar.activation(
                                y16[:, 4 * OW : 6 * OW], cp[:, 512 : 512 + 2 * OW],
                                AF.Copy,
                            )
                        for hi, hr in ((0, hr0), (1, hr1)):
                            nc.vector.bn_stats(
                                stats[:, si, :],
                                y16[:, hi * 4 * OW : (hi * 4 + hr) * OW],
                            )
                            si += 1
                        y3 = y16[:].rearrange("p (r w) -> p r w", w=OW)
                        v0 = (c0 + g0) // 2  # global row-pair index
                        npair = gn // 2
                        nc.vector.tensor_tensor(
                            vp[:, v0 : v0 + npair, :],
                            y3[:, 0 : gn : 2, :],
                            y3[:, 1 : gn : 2, :],
                            OP.max,
                        )
                        g0 += gn
                    # previous sample's postlude goes in after this sample's
                    # third chunk so its tiny aggregation matmul never makes
                    # the PE wait (an idle PE re-arms the HAM throttle)
                    if ci_ == 2 and pending is not None:
                        pending()
                        pending = None

                pending = make_postlude(b, stats, vp)
            pending()

    nc.finalize()
    _CACHED["nc"] = nc
    return nc


def _prep_consts(conv_w, conv_b, gn_w, gn_b, scale):
    w = np.asarray(conv_w, dtype=np.float32)
    gwsc = (np.asarray(gn_w, dtype=np.float32)
            * np.asarray(scale, dtype=np.float32).reshape(COUT))
    s = np.where(gwsc >= 0.0, np.float32(1.0), np.float32(-1.0))
    wsgn = (w * s[:, None, None, None]).astype(np.float16)
    wp = np.empty((3, 128, COUT), np.float16)
    ws = np.empty((3, 64, COUT), np.float16)
    for kw in range(3):
        wp[kw, 0:64, :] = wsgn[:, :, 0, kw].T
        wp[kw, 64:128, :] = wsgn[:, :, 1, kw].T
        ws[kw, :, :] = wsgn[:, :, 2, kw].T
    cb = np.asarray(conv_b, dtype=np.float32).reshape(COUT, 1)
    sg = s.reshape(COUT, 1)
    gws = gwsc.reshape(COUT, 1)
    gsa = np.abs(gwsc).reshape(COUT, 1)
    gbs = (np.asarray(gn_b, dtype=np.float32)
           * np.asarray(scale, dtype=np.float32).reshape(COUT)).reshape(COUT, 1)
    bones = np.zeros((COUT, COUT), np.float32)
    for g in range(NG):
        bones[g * GSZ : (g + 1) * GSZ, g * GSZ : (g + 1) * GSZ] = 1.0
    return wp, ws, cb, sg, gws, gsa, gbs, bones


def kernel(x, conv_w, conv_b, gn_w, gn_b, scale):
    x16 = np.ascontiguousarray(np.asarray(x)).astype(np.float16)
    wp, ws, cb, sg, gws, gsa, gbs, bones = _prep_consts(
        conv_w, conv_b, gn_w, gn_b, scale
    )
    nc = _build()
    in_maps = []
    for c in range(N_CORES):
        in_maps.append({
            "xs": x16[c * BPC : (c + 1) * BPC],
            "wp": wp, "ws": ws, "cb": cb, "sg": sg,
            "gws": gws, "gsa": gsa, "gbs": gbs, "bones": bones,
        })
    results = _run_cached(nc, in_maps)
    out = np.concatenate([results[c]["out"] for c in range(N_CORES)], axis=0)
    return np.ascontiguousarray(out.astype(np.float32))


def _run_cached(nc, in_maps):
    """run_bass_kernel_spmd's axon path with the jitted executable cached
    across calls (avoids re-tracing the shard_map wrapper every call)."""
    import jax
    import numpy as _np
    from jax.sharding import Mesh, PartitionSpec
    from jax.experimental.shard_map import shard_map
    from concourse import bass2jax

    if "runner" not in _CACHED:
        bass2jax.install_neuronx_cc_hook()
        partition_name = (
            nc.partition_id_tensor.name if nc.partition_id_tensor else None
        )
        in_names, out_names, out_avals, zero_outs = [], [], [], []
        for alloc in nc.m.functions[0].allocations:
            if not isinstance(alloc, mybir.MemoryLocationSet):
                continue
            name = alloc.memorylocations[0].name
            if alloc.kind == "ExternalInput":
                if name != partition_name:
                    in_names.append(name)
            elif alloc.kind == "ExternalOutput":
                shape = tuple(alloc.tensor_shape)
                dtype = mybir.dt.np(alloc.dtype)
                out_names.append(name)
                out_avals.append(jax.core.ShapedArray(shape, dtype))
                zero_outs.append(_np.zeros(shape, dtype))
        n_params = len(in_names)
        n_outs = len(out_avals)
        all_names = list(in_names) + list(out_names)
        if partition_name is not None:
            all_names.append(partition_name)
        donate = tuple(range(n_params, n_params + n_outs))

        def _body(*args):
            operands = list(args)
            if partition_name is not None:
                operands.append(bass2jax.partition_id_tensor())
            outs = bass2jax._bass_exec_p.bind(
                *operands,
                out_avals=tuple(out_avals),
                in_names=tuple(all_names),
                out_names=tuple(out_names),
                lowering_input_output_aliases=(),
                sim_require_finite=True,
                sim_require_nnan=True,
                nc=nc,
            )
            return tuple(outs)

        devices = jax.devices()[:N_CORES]
        mesh = Mesh(_np.asarray(devices), ("core",))
        in_specs = (PartitionSpec("core"),) * (n_params + n_outs)
        out_specs = (PartitionSpec("core"),) * n_outs
        sharded = jax.jit(
            shard_map(_body, mesh=mesh, in_specs=in_specs,
                      out_specs=out_specs, check_rep=False),
            donate_argnums=donate, keep_unused=True,
        )
        _CACHED["runner"] = (sharded, in_names, out_names, out_avals, zero_outs)

    sharded, in_names, out_names, out_avals, zero_outs = _CACHED["runner"]
    import numpy as _np2
    concat_in = [
        _np2.concatenate([_np2.asarray(in_maps[c][n]) for c in range(N_CORES)], axis=0)
        for n in in_names
    ]
    concat_zeros = [
        _np2.zeros((N_CORES * z.shape[0], *z.shape[1:]), z.dtype) for z in zero_outs
    ]
    out_arrs = sharded(*concat_in, *concat_zeros)
    return [
        {
            name: _np2.asarray(out_arrs[i]).reshape(N_CORES, *out_avals[i].shape)[c]
            for i, name in enumerate(out_names)
        }
        for c in range(N_CORES)
    ]


if __name__ == "__main__":
    rng = np.random.default_rng(0)
    x = rng.standard_normal((B_FULL, CIN, H, W), dtype=np.float32)
    cw = rng.standard_normal((COUT, CIN, 3, 3), dtype=np.float32)
    out = kernel(x, cw, rng.standard_normal(COUT, dtype=np.float32),
                 rng.standard_normal(COUT, dtype=np.float32),
                 rng.standard_normal(COUT, dtype=np.float32),
                 rng.standard_normal((COUT, 1, 1), dtype=np.float32))
    print(out.shape, out.dtype)


# revision 26
# speedup vs baseline: 1.0065x; 1.0065x over previous
"""Fused conv3x3 -> GroupNorm(16) -> channel scale -> maxpool2x2 -> clamp[0,1]
Trainium2 Bass kernel, data-parallel over batch on 8 NeuronCores.

Input  x [32, 64, 128, 128] f32  -> output [32, 128, 63, 63] f32.
Each core handles 4 samples.

Conv in fp16 (1 cycle/column on the PE vs 3 for fp32-HIGH; input rounding
error after GroupNorm is ~5e-3, well inside the 2e-2 gate). Tap-wise
matmuls: partition p = ci + 64*r holds x[ci, row+r, :] (r in {0,1}), so one
matmul with a [128, 128] stacked weight covers taps (kh=0, kw) and (kh=1,
kw) at once; kh=2 taps run as 64-partition matmuls off the low half.
6 matmuls per 4 output rows.

Post-processing pools BEFORE the GroupNorm affine: sign(gn_w*scale) is
folded into the conv weights on the host so the affine slope A is always
>= 0, making maxpool commute with it. Per 8-row PSUM chunk: ACT evacuates
to fp16, DVE takes bn_stats straight from PSUM, GPSIMD maxes row pairs;
once per sample GPSIMD maxes column pairs, ACT applies A*y+B with fused
Relu on the pooled 63x63 tile, GPSIMD clamps the top at 1.
"""

import numpy as np

import concourse.bacc as bacc
import concourse.mybir as mybir
import concourse.tile as tile

N_CORES = 8
B_FULL, CIN, H, W = 32, 64, 128, 128
COUT = 128
BPC = B_FULL // N_CORES  # samples per core
OH = OW = 126
PH = PW = 63
NG = 16  # groups
GSZ = COUT // NG  # 8 channels per group
EPS = 1e-5

# (out_row0, n_out_rows) conv chunks; x rows [r0 .. r0+nor+1] feed each
CHUNKS = [(16 * i, 16) for i in range(7)] + [(112, 14)]
NSTATS = 32  # bn_stats slots per sample

_CACHED = {}


def _build():
    if "nc" in _CACHED:
        return _CACHED["nc"]
    f32 = mybir.dt.float32
    f16 = mybir.dt.float16
    AF = mybir.ActivationFunctionType
    OP = mybir.AluOpType

    nc = bacc.Bacc("TRN2", target_bir_lowering=False, debug=False)
    xs = nc.dram_tensor("xs", [BPC, CIN, H, W], f16, kind="ExternalInput").ap()
    wp_d = nc.dram_tensor("wp", [3, 128, COUT], f16, kind="ExternalInput").ap()
    ws_d = nc.dram_tensor("ws", [3, 64, COUT], f16, kind="ExternalInput").ap()
    cb_d = nc.dram_tensor("cb", [COUT, 1], f32, kind="ExternalInput").ap()
    sg_d = nc.dram_tensor("sg", [COUT, 1], f32, kind="ExternalInput").ap()
    gws_d = nc.dram_tensor("gws", [COUT, 1], f32, kind="ExternalInput").ap()
    gsa_d = nc.dram_tensor("gsa", [COUT, 1], f32, kind="ExternalInput").ap()
    gbs_d = nc.dram_tensor("gbs", [COUT, 1], f32, kind="ExternalInput").ap()
    bones_d = nc.dram_tensor("bones", [COUT, COUT], f32, kind="ExternalInput").ap()
    out_d = nc.dram_tensor("out", [BPC, COUT, PH, PW], f16, kind="ExternalOutput").ap()

    with tile.TileContext(nc) as tc:
        with (
            tc.tile_pool(name="consts", bufs=1) as cpool,
            tc.tile_pool(name="xpool", bufs=3) as xpool,
            tc.tile_pool(name="ypool", bufs=4) as ypool,
            tc.tile_pool(name="vpool", bufs=2) as vpool,
            tc.tile_pool(name="hpool", bufs=2) as hpool,
            tc.tile_pool(name="zpool", bufs=2) as zpool,
            tc.tile_pool(name="stpool", bufs=2) as stpool,
            tc.tile_pool(name="cps", bufs=3, space="PSUM") as cps,
            tc.tile_pool(name="gps", bufs=1, space="PSUM") as gps,
        ):
            wp = cpool.tile([128, 3 * COUT], f16, name="wp_t")
            ws = cpool.tile([64, 3 * COUT], f16, name="ws_t")
            for kw in range(3):
                nc.sync.dma_start(wp[:, kw * COUT : (kw + 1) * COUT], wp_d[kw])
                nc.sync.dma_start(ws[:, kw * COUT : (kw + 1) * COUT], ws_d[kw])
            cb = cpool.tile([COUT, 1], f32, name="cb_t")
            nc.sync.dma_start(cb[:], cb_d[:])
            sg = cpool.tile([COUT, 1], f32, name="sg_t")
            nc.sync.dma_start(sg[:], sg_d[:])
            gws = cpool.tile([COUT, 1], f32, name="gws_t")
            nc.sync.dma_start(gws[:], gws_d[:])
            gsa = cpool.tile([COUT, 1], f32, name="gsa_t")
            nc.sync.dma_start(gsa[:], gsa_d[:])
            gbs = cpool.tile([COUT, 1], f32, name="gbs_t")
            nc.sync.dma_start(gbs[:], gbs_d[:])
            bones = cpool.tile([COUT, COUT], f32, name="bones_t")
            nc.sync.dma_start(bones[:], bones_d[:])
            zeros1 = cpool.tile([COUT, 1], f32, name="zeros1")
            nc.vector.memset(zeros1[:], 0.0)
            epst = cpool.tile([COUT, 1], f32, name="epst")
            nc.vector.memset(epst[:], EPS)

            # Warmup: keep the PE busy while the first x chunk loads so the
            # HAM clock-gate reaches 8/8 before the real stream starts.
            # Depends only on a memset, so it starts as soon as the PE is up.
            wmov = cpool.tile([128, 384], f16, name="wmov")
            nc.vector.memset(wmov[:], 0.0)
            wps = gps.tile([128, 384], f32, tag="warm", name="warm_ps")
            NWARM = 30
            for i in range(NWARM):
                nc.tensor.matmul(
                    wps[:], wmov[:, 0:COUT], wmov[:],
                    start=(i == 0), stop=(i == NWARM - 1),
                )
            wjunk = cpool.tile([128, 1], f32, name="wjunk")
            nc.vector.tensor_reduce(
                wjunk[:], wps[:], mybir.AxisListType.X, OP.max
            )

            def make_postlude(b, stats, vp):
                def postlude():
                    # per-channel -> per-group affine coefficients (mostly
                    # on the Scalar engine; [128,1] ops via bias/scale APs)
                    mv = stpool.tile([128, 2], f32, tag="mv", name="mv")
                    nc.vector.bn_aggr(mv[:], stats[:])
                    st = stpool.tile([128, 2], f32, tag="sts", name="st")
                    # t1 = sg*mean' + cb ; t2 = var' + t1^2 — all on the
                    # Vector engine right after bn_aggr (no cross-engine
                    # hops on the critical path to the aggregation matmul)
                    nc.vector.tensor_scalar(
                        st[:, 0:1], mv[:, 0:1], sg[:], cb[:], OP.mult, OP.add
                    )
                    t1sq = stpool.tile([128, 1], f32, tag="t1sq", name="t1sq")
                    nc.vector.tensor_tensor(
                        t1sq[:], st[:, 0:1], st[:, 0:1], OP.mult
                    )
                    nc.vector.tensor_tensor(
                        st[:, 1:2], mv[:, 1:2], t1sq[:], OP.add
                    )
                    gsum = gps.tile([128, 2], f32, tag="gsum", name="gsum")
                    nc.tensor.matmul(gsum[:], bones[:], st[:], start=True,
                                     stop=True)
                    mgrp = stpool.tile([128, 1], f32, tag="mgrp", name="mgrp")
                    nc.scalar.activation(
                        mgrp[:], gsum[:, 0:1], AF.Copy, scale=1.0 / GSZ
                    )
                    vgrp = stpool.tile([128, 1], f32, tag="vgrp", name="vgrp")
                    nc.scalar.activation(
                        vgrp[:], gsum[:, 1:2], AF.Identity, bias=epst[:],
                        scale=1.0 / GSZ
                    )
                    msq = stpool.tile([128, 1], f32, tag="msq", name="msq")
                    nc.scalar.activation(msq[:], mgrp[:], AF.Square)
                    nc.vector.tensor_tensor(vgrp[:], vgrp[:], msq[:],
                                            OP.subtract)
                    sdev = stpool.tile([128, 1], f32, tag="sdev", name="sdev")
                    nc.scalar.activation(sdev[:], vgrp[:], AF.Sqrt,
                                         bias=zeros1[:])
                    inv = stpool.tile([128, 1], f32, tag="inv", name="inv")
                    nc.vector.reciprocal(inv[:], sdev[:])
                    # Apos = inv*|gws| (>=0) ; B2 = inv*gws*(cb-mgrp) + gbs
                    apos = stpool.tile([128, 1], f32, tag="Ap", name="apos")
                    nc.scalar.activation(apos[:], gsa[:], AF.Copy,
                                         scale=inv[:])
                    sa = stpool.tile([128, 1], f32, tag="sA", name="sa")
                    nc.scalar.activation(sa[:], gws[:], AF.Copy, scale=inv[:])
                    b2 = stpool.tile([128, 1], f32, tag="B2", name="b2")
                    nc.vector.tensor_tensor(b2[:], cb[:], mgrp[:], OP.subtract)
                    nc.scalar.activation(b2[:], b2[:], AF.Identity,
                                         bias=gbs[:], scale=sa[:])

                    # column-pair max, then affine+clamp: first row-half on
                    # the Scalar engine, second on the Vector engine so the
                    # two halves finish in parallel
                    hp = hpool.tile([128, PH, PW], f16, tag="hp", name="hp")
                    zout = zpool.tile([128, PH * PW], f16, tag="z", name="zout")
                    halves = ((0, 32), (32, PH))
                    for r0, r1 in halves:
                        nc.vector.tensor_tensor(
                            hp[:, r0:r1, :],
                            vp[:, r0:r1, 0 : OW : 2],
                            vp[:, r0:r1, 1 : OW : 2],
                            OP.max
                        )
                    r0, r1 = halves[0]
                    seg0 = zout[:, r0 * PW : r1 * PW]
                    nc.scalar.activation(
                        seg0.rearrange("p (a b) -> p a b", b=PW),
                        hp[:, r0:r1, :], AF.Relu, bias=b2[:], scale=apos[:]
                    )
                    nc.vector.tensor_scalar(seg0, seg0, 1.0, None, OP.min)
                    r0, r1 = halves[1]
                    seg1 = zout[:, r0 * PW : r1 * PW]
                    nc.vector.tensor_scalar(
                        seg1.rearrange("p (a b) -> p a b", b=PW),
                        hp[:, r0:r1, :], apos[:], b2[:], OP.mult, OP.add
                    )
                    nc.vector.tensor_scalar(seg1, seg1, 1.0, 0.0, OP.min,
                                            OP.max)
                    for r0, r1 in halves:
                        nc.sync.dma_start(
                            out_d[b, :, r0:r1, :].rearrange("c h w -> c (h w)"),
                            zout[:, r0 * PW : r1 * PW],
                        )
                return postlude

            pending = None
            for b in range(BPC):
                stats = stpool.tile([128, NSTATS, 6], f32, tag="st", name="stats")
                vp = vpool.tile([128, PH, OW], f16, tag="vp", name="vp")
                si = 0
                for ci_, (c0, nor) in enumerate(CHUNKS):
                    xt = xpool.tile([128, 18, W], f16, tag="x", name="xt")
                    # low half: rows c0 .. c0+nor+1 (kh0 + source of kh2 taps);
                    # the very first load is split across dynamic DMA queues
                    # so the conv stream can start sooner
                    if b == 0 and ci_ == 0:
                        for s0, s1 in ((0, 6), (6, 12), (12, nor + 2)):
                            nc.gpsimd.dma_start(
                                xt[0:64, s0:s1, :], xs[b, :, c0 + s0 : c0 + s1, :]
                            )
                    else:
                        nc.gpsimd.dma_start(
                            xt[0:64, 0 : nor + 2, :],
                            xs[b, :, c0 : c0 + nor + 2, :]
                        )
                    # high half: rows c0+1 .. c0+nor (kh1 taps), copied
                    # SBUF->SBUF from the low half (saves HBM reads); the
                    # first chunk reads HBM directly so it doesn't serialize
                    # behind the low half
                    if b == 0 and ci_ == 0:
                        nc.gpsimd.dma_start(
                            xt[64:128, 0:nor, :],
                            xs[b, :, c0 + 1 : c0 + 1 + nor, :]
                        )
                    else:
                        nc.gpsimd.dma_start(
                            xt[64:128, 0:nor, :], xt[0:64, 1 : 1 + nor, :]
                        )

                    g0 = 0  # chunk-local output row
                    while g0 < nor:
                        gn = min(8, nor - g0)  # 8 or 6 (last chunk tail)
                        hr0, hr1 = (4, 4) if gn == 8 else (4, 2)
                        cp = cps.tile([128, 1024], f32, tag="cp", name="cp")
                        for hi, hr in ((0, hr0), (1, hr1)):
                            l0 = g0 + hi * 4
                            outap = cp[:, hi * 512 : hi * 512 + hr * OW]
                            for kw in range(3):
                                nc.tensor.matmul(
                                    outap,
                                    wp[:, kw * COUT : (kw + 1) * COUT],
                                    xt[:, l0 : l0 + hr, kw : kw + OW],
                                    start=(kw == 0),
                                    stop=False,
                                )
                            for kw in range(3):
                                nc.tensor.matmul(
                                    outap,
                                    ws[:, kw * COUT : (kw + 1) * COUT],
                                    xt[0:64, l0 + 2 : l0 + 2 + hr, kw : kw + OW],
                                    start=False,
                                    stop=(kw == 2),
                                )
                        # evacuate to fp16 (ACT), stats from PSUM (DVE),
                        # row-pair max (GPSIMD)
                        y16 = ypool.tile([128, 8 * OW], f16, tag="y", name="y16")
                        cpv = cp[:].rearrange("p (h w) -> p h w", w=512)
                        if gn == 8:
                            nc.scalar.activation(
                                y16[:].rearrange("p (h w) -> p h w", w=4 * OW),
                                cpv[:, :, 0 : 4 * OW],
                                AF.Copy,
                            )
                        else:
                            nc.scalar.activation(
                                y16[:, 0 : 4 * OW], cp[:, 0 : 4 * OW], AF.Copy
                            )
                            nc.scalar.activation(
                                y16[:, 4 * OW : 6 * OW], cp[:, 512 : 512 + 2 * OW],
                                AF.Copy,
                            )
                        for hi, hr in ((0, hr0), (1, hr1)):
                            nc.vector.bn_stats(
                                stats[:, si, :],
                                y16[:, hi * 4 * OW : (hi * 4 + hr) * OW],
                            )
                            si += 1
                        y3 = y16[:].rearrange("p (r w) -> p r w", w=OW)
                        v0 = (c0 + g0) // 2  # global row-pair index
                        npair = gn // 2
                        nc.vector.tensor_tensor(
                            vp[:, v0 : v0 + npair, :],
                            y3[:, 0 : gn : 2, :],
                            y3[:, 1 : gn : 2, :],
                            OP.max,
                        )
                        g0 += gn
                    # previous sample's postlude goes in after this sample's
                    # third chunk so its tiny aggregation matmul never makes
                    # the PE wait (an idle PE re-arms the HAM throttle)
                    if ci_ == 2 and pending is not None:
                        pending()
                        pending = None

                pending = make_postlude(b, stats, vp)
            pending()

    nc.finalize()
    _CACHED["nc"] = nc
    return nc


def _prep_consts(conv_w, conv_b, gn_w, gn_b, scale):
    w = np.asarray(conv_w, dtype=np.float32)
    gwsc = (np.asarray(gn_w, dtype=np.float32)
            * np.asarray(scale, dtype=np.float32).reshape(COUT))
    s = np.where(gwsc >= 0.0, np.float32(1.0), np.float32(-1.0))
    wsgn = (w * s[:, None, None, None]).astype(np.float16)
    wp = np.empty((3, 128, COUT), np.float16)
    ws = np.empty((3, 64, COUT), np.float16)
    for kw in range(3):
        wp[kw, 0:64, :] = wsgn[:, :, 0, kw].T
        wp[kw, 64:128, :] = wsgn[:, :, 1, kw].T
        ws[kw, :, :] = wsgn[:, :, 2, kw].T
    cb = np.asarray(conv_b, dtype=np.float32).reshape(COUT, 1)
    sg = s.reshape(COUT, 1)
    gws = gwsc.reshape(COUT, 1)
    gsa = np.abs(gwsc).reshape(COUT, 1)
    gbs = (np.asarray(gn_b, dtype=np.float32)
           * np.asarray(scale, dtype=np.float32).reshape(COUT)).reshape(COUT, 1)
    bones = np.zeros((COUT, COUT), np.float32)
    for g in range(NG):
        bones[g * GSZ : (g + 1) * GSZ, g * GSZ : (g + 1) * GSZ] = 1.0
    return wp, ws, cb, sg, gws, gsa, gbs, bones


def kernel(x, conv_w, conv_b, gn_w, gn_b, scale):
    x16 = np.ascontiguousarray(np.asarray(x)).astype(np.float16)
    wp, ws, cb, sg, gws, gsa, gbs, bones = _prep_consts(
        conv_w, conv_b, gn_w, gn_b, scale
    )
    nc = _build()
    in_maps = []
    for c in range(N_CORES):
        in_maps.append({
            "xs": x16[c * BPC : (c + 1) * BPC],
            "wp": wp, "ws": ws, "cb": cb, "sg": sg,
            "gws": gws, "gsa": gsa, "gbs": gbs, "bones": bones,
        })
    results = _run_cached(nc, in_maps)
    out = np.concatenate([results[c]["out"] for c in range(N_CORES)], axis=0)
    return np.ascontiguousarray(out.astype(np.float32))


def _run_cached(nc, in_maps):
    """run_bass_kernel_spmd's axon path with the jitted executable cached
    across calls (avoids re-tracing the shard_map wrapper every call)."""
    import jax
    import numpy as _np
    from jax.sharding import Mesh, PartitionSpec
    from jax.experimental.shard_map import shard_map
    from concourse import bass2jax

    if "runner" not in _CACHED:
        bass2jax.install_neuronx_cc_hook()
        partition_name = (
            nc.partition_id_tensor.name if nc.partition_id_tensor else None
        )
        in_names, out_names, out_avals, zero_outs = [], [], [], []
        for alloc in nc.m.functions[0].allocations:
            if not isinstance(alloc, mybir.MemoryLocationSet):
                continue
            name = alloc.memorylocations[0].name
            if alloc.kind == "ExternalInput":
                if name != partition_name:
                    in_names.append(name)
            elif alloc.kind == "ExternalOutput":
                shape = tuple(alloc.tensor_shape)
                dtype = mybir.dt.np(alloc.dtype)
                out_names.append(name)
                out_avals.append(jax.core.ShapedArray(shape, dtype))
                zero_outs.append(_np.zeros(shape, dtype))
        n_params = len(in_names)
        n_outs = len(out_avals)
        all_names = list(in_names) + list(out_names)
        if partition_name is not None:
            all_names.append(partition_name)
        donate = tuple(range(n_params, n_params + n_outs))

        def _body(*args):
            operands = list(args)
            if partition_name is not None:
                operands.append(bass2jax.partition_id_tensor())
            outs = bass2jax._bass_exec_p.bind(
                *operands,
                out_avals=tuple(out_avals),
                in_names=tuple(all_names),
                out_names=tuple(out_names),
                lowering_input_output_aliases=(),
                sim_require_finite=True,
                sim_require_nnan=True,
                nc=nc,
            )
            return tuple(outs)

        devices = jax.devices()[:N_CORES]
        mesh = Mesh(_np.asarray(devices), ("core",))
        in_specs = (PartitionSpec("core"),) * (n_params + n_outs)
        out_specs = (PartitionSpec("core"),) * n_outs
        sharded = jax.jit(
            shard_map(_body, mesh=mesh, in_specs=in_specs,
                      out_specs=out_specs, check_rep=False),
            donate_argnums=donate, keep_unused=True,
        )
        _CACHED["runner"] = (sharded, in_names, out_names, out_avals, zero_outs)

    sharded, in_names, out_names, out_avals, zero_outs = _CACHED["runner"]
    import numpy as _np2
    concat_in = [
        _np2.concatenate([_np2.asarray(in_maps[c][n]) for c in range(N_CORES)], axis=0)
        for n in in_names
    ]
    concat_zeros = [
        _np2.zeros((N_CORES * z.shape[0], *z.shape[1:]), z.dtype) for z in zero_outs
    ]
    out_arrs = sharded(*concat_in, *concat_zeros)
    return [
        {
            name: _np2.asarray(out_arrs[i]).reshape(N_CORES, *out_avals[i].shape)[c]
            for i, name in enumerate(out_names)
        }
        for c in range(N_CORES)
    ]


if __name__ == "__main__":
    rng = np.random.default_rng(0)
    x = rng.standard_normal((B_FULL, CIN, H, W), dtype=np.float32)
    cw = rng.standard_normal((COUT, CIN, 3, 3), dtype=np.float32)
    out = kernel(x, cw, rng.standard_normal(COUT, dtype=np.float32),
                 rng.standard_normal(COUT, dtype=np.float32),
                 rng.standard_normal(COUT, dtype=np.float32),
                 rng.standard_normal((COUT, 1, 1), dtype=np.float32))
    print(out.shape, out.dtype)


# revision 27
# speedup vs baseline: 1.0144x; 1.0079x over previous
"""Fused conv3x3 -> GroupNorm(16) -> channel scale -> maxpool2x2 -> clamp[0,1]
Trainium2 Bass kernel, data-parallel over batch on 8 NeuronCores.

Input  x [32, 64, 128, 128] f32  -> output [32, 128, 63, 63] f32.
Each core handles 4 samples.

Conv in fp16 (1 cycle/column on the PE vs 3 for fp32-HIGH; input rounding
error after GroupNorm is ~5e-3, well inside the 2e-2 gate). Tap-wise
matmuls: partition p = ci + 64*r holds x[ci, row+r, :] (r in {0,1}), so one
matmul with a [128, 128] stacked weight covers taps (kh=0, kw) and (kh=1,
kw) at once; kh=2 taps run as 64-partition matmuls off the low half.
6 matmuls per 4 output rows.

Post-processing pools BEFORE the GroupNorm affine: sign(gn_w*scale) is
folded into the conv weights on the host so the affine slope A is always
>= 0, making maxpool commute with it. Per 8-row PSUM chunk: ACT evacuates
to fp16, DVE takes bn_stats straight from PSUM, GPSIMD maxes row pairs;
once per sample GPSIMD maxes column pairs, ACT applies A*y+B with fused
Relu on the pooled 63x63 tile, GPSIMD clamps the top at 1.
"""

import numpy as np

import concourse.bacc as bacc
import concourse.mybir as mybir
import concourse.tile as tile

N_CORES = 8
B_FULL, CIN, H, W = 32, 64, 128, 128
COUT = 128
BPC = B_FULL // N_CORES  # samples per core
OH = OW = 126
PH = PW = 63
NG = 16  # groups
GSZ = COUT // NG  # 8 channels per group
EPS = 1e-5

# (out_row0, n_out_rows) conv chunks; x rows [r0 .. r0+nor+1] feed each
CHUNKS = [(16 * i, 16) for i in range(7)] + [(112, 14)]
NSTATS = 32  # bn_stats slots per sample

_CACHED = {}


def _build():
    if "nc" in _CACHED:
        return _CACHED["nc"]
    f32 = mybir.dt.float32
    f16 = mybir.dt.float16
    AF = mybir.ActivationFunctionType
    OP = mybir.AluOpType

    nc = bacc.Bacc("TRN2", target_bir_lowering=False, debug=False)
    xs = nc.dram_tensor("xs", [BPC, CIN, H, W], f16, kind="ExternalInput").ap()
    wp_d = nc.dram_tensor("wp", [3, 128, COUT], f16, kind="ExternalInput").ap()
    ws_d = nc.dram_tensor("ws", [3, 64, COUT], f16, kind="ExternalInput").ap()
    cb_d = nc.dram_tensor("cb", [COUT, 1], f32, kind="ExternalInput").ap()
    sg_d = nc.dram_tensor("sg", [COUT, 1], f32, kind="ExternalInput").ap()
    gws_d = nc.dram_tensor("gws", [COUT, 1], f32, kind="ExternalInput").ap()
    gsa_d = nc.dram_tensor("gsa", [COUT, 1], f32, kind="ExternalInput").ap()
    gbs_d = nc.dram_tensor("gbs", [COUT, 1], f32, kind="ExternalInput").ap()
    bones_d = nc.dram_tensor("bones", [COUT, COUT], f32, kind="ExternalInput").ap()
    out_d = nc.dram_tensor("out", [BPC, COUT, PH, PW], f16, kind="ExternalOutput").ap()

    with tile.TileContext(nc) as tc:
        with (
            tc.tile_pool(name="consts", bufs=1) as cpool,
            tc.tile_pool(name="xpool", bufs=3) as xpool,
            tc.tile_pool(name="ypool", bufs=4) as ypool,
            tc.tile_pool(name="vpool", bufs=2) as vpool,
            tc.tile_pool(name="hpool", bufs=2) as hpool,
            tc.tile_pool(name="zpool", bufs=2) as zpool,
            tc.tile_pool(name="stpool", bufs=2) as stpool,
            tc.tile_pool(name="cps", bufs=3, space="PSUM") as cps,
            tc.tile_pool(name="gps", bufs=1, space="PSUM") as gps,
        ):
            wp = cpool.tile([128, 3 * COUT], f16, name="wp_t")
            ws = cpool.tile([64, 3 * COUT], f16, name="ws_t")
            for kw in range(3):
                nc.sync.dma_start(wp[:, kw * COUT : (kw + 1) * COUT], wp_d[kw])
                nc.sync.dma_start(ws[:, kw * COUT : (kw + 1) * COUT], ws_d[kw])
            cb = cpool.tile([COUT, 1], f32, name="cb_t")
            nc.sync.dma_start(cb[:], cb_d[:])
            sg = cpool.tile([COUT, 1], f32, name="sg_t")
            nc.sync.dma_start(sg[:], sg_d[:])
            gws = cpool.tile([COUT, 1], f32, name="gws_t")
            nc.sync.dma_start(gws[:], gws_d[:])
            gsa = cpool.tile([COUT, 1], f32, name="gsa_t")
            nc.sync.dma_start(gsa[:], gsa_d[:])
            gbs = cpool.tile([COUT, 1], f32, name="gbs_t")
            nc.sync.dma_start(gbs[:], gbs_d[:])
            bones = cpool.tile([COUT, COUT], f32, name="bones_t")
            nc.sync.dma_start(bones[:], bones_d[:])
            zeros1 = cpool.tile([COUT, 1], f32, name="zeros1")
            nc.vector.memset(zeros1[:], 0.0)
            epst = cpool.tile([COUT, 1], f32, name="epst")
            nc.vector.memset(epst[:], EPS)

            # Warmup: keep the PE busy while the first x chunk loads so the
            # HAM clock-gate reaches 8/8 before the real stream starts.
            # Depends only on a memset, so it starts as soon as the PE is up.
            wmov = cpool.tile([128, 384], f16, name="wmov")
            nc.vector.memset(wmov[:], 0.0)
            wps = gps.tile([128, 384], f32, tag="warm", name="warm_ps")
            NWARM = 30
            for i in range(NWARM):
                nc.tensor.matmul(
                    wps[:], wmov[:, 0:COUT], wmov[:],
                    start=(i == 0), stop=(i == NWARM - 1),
                )
            wjunk = cpool.tile([128, 1], f32, name="wjunk")
            nc.vector.tensor_reduce(
                wjunk[:], wps[:], mybir.AxisListType.X, OP.max
            )

            def make_postlude(b, stats, vp):
                def postlude():
                    # per-channel -> per-group affine coefficients (mostly
                    # on the Scalar engine; [128,1] ops via bias/scale APs)
                    mv = stpool.tile([128, 2], f32, tag="mv", name="mv")
                    nc.vector.bn_aggr(mv[:], stats[:])
                    st = stpool.tile([128, 2], f32, tag="sts", name="st")
                    # t1 = sg*mean' + cb ; t2 = var' + t1^2 (Scalar engine —
                    # it is idler than Vector at sample boundaries)
                    nc.scalar.activation(
                        st[:, 0:1], mv[:, 0:1], AF.Identity,
                        bias=cb[:], scale=sg[:]
                    )
                    t1sq = stpool.tile([128, 1], f32, tag="t1sq", name="t1sq")
                    nc.scalar.activation(t1sq[:], st[:, 0:1], AF.Square)
                    nc.scalar.activation(
                        st[:, 1:2], mv[:, 1:2], AF.Identity, bias=t1sq[:]
                    )
                    gsum = gps.tile([128, 2], f32, tag="gsum", name="gsum")
                    nc.tensor.matmul(gsum[:], bones[:], st[:], start=True,
                                     stop=True)
                    mgrp = stpool.tile([128, 1], f32, tag="mgrp", name="mgrp")
                    nc.scalar.activation(
                        mgrp[:], gsum[:, 0:1], AF.Copy, scale=1.0 / GSZ
                    )
                    vgrp = stpool.tile([128, 1], f32, tag="vgrp", name="vgrp")
                    nc.scalar.activation(
                        vgrp[:], gsum[:, 1:2], AF.Identity, bias=epst[:],
                        scale=1.0 / GSZ
                    )
                    msq = stpool.tile([128, 1], f32, tag="msq", name="msq")
                    nc.scalar.activation(msq[:], mgrp[:], AF.Square)
                    nc.vector.tensor_tensor(vgrp[:], vgrp[:], msq[:],
                                            OP.subtract)
                    sdev = stpool.tile([128, 1], f32, tag="sdev", name="sdev")
                    nc.scalar.activation(sdev[:], vgrp[:], AF.Sqrt,
                                         bias=zeros1[:])
                    inv = stpool.tile([128, 1], f32, tag="inv", name="inv")
                    nc.vector.reciprocal(inv[:], sdev[:])
                    # Apos = inv*|gws| (>=0) ; B2 = inv*gws*(cb-mgrp) + gbs
                    apos = stpool.tile([128, 1], f32, tag="Ap", name="apos")
                    nc.scalar.activation(apos[:], gsa[:], AF.Copy,
                                         scale=inv[:])
                    sa = stpool.tile([128, 1], f32, tag="sA", name="sa")
                    nc.scalar.activation(sa[:], gws[:], AF.Copy, scale=inv[:])
                    b2 = stpool.tile([128, 1], f32, tag="B2", name="b2")
                    nc.vector.tensor_tensor(b2[:], cb[:], mgrp[:], OP.subtract)
                    nc.scalar.activation(b2[:], b2[:], AF.Identity,
                                         bias=gbs[:], scale=sa[:])

                    # column-pair max, then affine+clamp: first row-half on
                    # the Scalar engine, second on the Vector engine so the
                    # two halves finish in parallel
                    hp = hpool.tile([128, PH, PW], f16, tag="hp", name="hp")
                    zout = zpool.tile([128, PH * PW], f16, tag="z", name="zout")
                    halves = ((0, 32), (32, PH))
                    for r0, r1 in halves:
                        nc.vector.tensor_tensor(
                            hp[:, r0:r1, :],
                            vp[:, r0:r1, 0 : OW : 2],
                            vp[:, r0:r1, 1 : OW : 2],
                            OP.max
                        )
                    r0, r1 = halves[0]
                    seg0 = zout[:, r0 * PW : r1 * PW]
                    nc.scalar.activation(
                        seg0.rearrange("p (a b) -> p a b", b=PW),
                        hp[:, r0:r1, :], AF.Relu, bias=b2[:], scale=apos[:]
                    )
                    nc.vector.tensor_scalar(seg0, seg0, 1.0, None, OP.min)
                    r0, r1 = halves[1]
                    seg1 = zout[:, r0 * PW : r1 * PW]
                    nc.vector.tensor_scalar(
                        seg1.rearrange("p (a b) -> p a b", b=PW),
                        hp[:, r0:r1, :], apos[:], b2[:], OP.mult, OP.add
                    )
                    nc.vector.tensor_scalar(seg1, seg1, 1.0, 0.0, OP.min,
                                            OP.max)
                    for r0, r1 in halves:
                        nc.sync.dma_start(
                            out_d[b, :, r0:r1, :].rearrange("c h w -> c (h w)"),
                            zout[:, r0 * PW : r1 * PW],
                        )
                return postlude

            pending = None
            for b in range(BPC):
                stats = stpool.tile([128, NSTATS, 6], f32, tag="st", name="stats")
                vp = vpool.tile([128, PH, OW], f16, tag="vp", name="vp")
                si = 0
                for ci_, (c0, nor) in enumerate(CHUNKS):
                    xt = xpool.tile([128, 18, W], f16, tag="x", name="xt")
                    # low half: rows c0 .. c0+nor+1 (kh0 + source of kh2 taps);
                    # the very first load is split across dynamic DMA queues
                    # so the conv stream can start sooner
                    if b == 0 and ci_ == 0:
                        for s0, s1 in ((0, 6), (6, 12), (12, nor + 2)):
                            nc.gpsimd.dma_start(
                                xt[0:64, s0:s1, :], xs[b, :, c0 + s0 : c0 + s1, :]
                            )
                    else:
                        nc.gpsimd.dma_start(
                            xt[0:64, 0 : nor + 2, :],
                            xs[b, :, c0 : c0 + nor + 2, :]
                        )
                    # high half: rows c0+1 .. c0+nor (kh1 taps), copied
                    # SBUF->SBUF from the low half (saves HBM reads); the
                    # first chunk reads HBM directly so it doesn't serialize
                    # behind the low half
                    if b == 0 and ci_ == 0:
                        nc.gpsimd.dma_start(
                            xt[64:128, 0:nor, :],
                            xs[b, :, c0 + 1 : c0 + 1 + nor, :]
                        )
                    else:
                        nc.gpsimd.dma_start(
                            xt[64:128, 0:nor, :], xt[0:64, 1 : 1 + nor, :]
                        )

                    g0 = 0  # chunk-local output row
                    while g0 < nor:
                        gn = min(8, nor - g0)  # 8 or 6 (last chunk tail)
                        hr0, hr1 = (4, 4) if gn == 8 else (4, 2)
                        cp = cps.tile([128, 1024], f32, tag="cp", name="cp")
                        for hi, hr in ((0, hr0), (1, hr1)):
                            l0 = g0 + hi * 4
                            outap = cp[:, hi * 512 : hi * 512 + hr * OW]
                            for kw in range(3):
                                nc.tensor.matmul(
                                    outap,
                                    wp[:, kw * COUT : (kw + 1) * COUT],
                                    xt[:, l0 : l0 + hr, kw : kw + OW],
                                    start=(kw == 0),
                                    stop=False,
                                )
                            for kw in range(3):
                                nc.tensor.matmul(
                                    outap,
                                    ws[:, kw * COUT : (kw + 1) * COUT],
                                    xt[0:64, l0 + 2 : l0 + 2 + hr, kw : kw + OW],
                                    start=False,
                                    stop=(kw == 2),
                                )
                        # evacuate to fp16 (ACT), stats from PSUM (DVE),
                        # row-pair max (GPSIMD)
                        y16 = ypool.tile([128, 8 * OW], f16, tag="y", name="y16")
                        cpv = cp[:].rearrange("p (h w) -> p h w", w=512)
                        if gn == 8:
                            nc.scalar.activation(
                                y16[:].rearrange("p (h w) -> p h w", w=4 * OW),
                                cpv[:, :, 0 : 4 * OW],
                                AF.Copy,
                            )
                        else:
                            nc.scalar.activation(
                                y16[:, 0 : 4 * OW], cp[:, 0 : 4 * OW], AF.Copy
                            )
                            nc.scalar.activation(
                                y16[:, 4 * OW : 6 * OW], cp[:, 512 : 512 + 2 * OW],
                                AF.Copy,
                            )
                        for hi, hr in ((0, hr0), (1, hr1)):
                            nc.vector.bn_stats(
                                stats[:, si, :],
                                y16[:, hi * 4 * OW : (hi * 4 + hr) * OW],
                            )
                            si += 1
                        y3 = y16[:].rearrange("p (r w) -> p r w", w=OW)
                        v0 = (c0 + g0) // 2  # global row-pair index
                        npair = gn // 2
                        nc.vector.tensor_tensor(
                            vp[:, v0 : v0 + npair, :],
                            y3[:, 0 : gn : 2, :],
                            y3[:, 1 : gn : 2, :],
                            OP.max,
                        )
                        g0 += gn
                    # previous sample's postlude goes in after this sample's
                    # third chunk so its tiny aggregation matmul never makes
                    # the PE wait (an idle PE re-arms the HAM throttle)
                    if ci_ == 2 and pending is not None:
                        pending()
                        pending = None

                pending = make_postlude(b, stats, vp)
            pending()

    nc.finalize()
    _CACHED["nc"] = nc
    return nc


def _prep_consts(conv_w, conv_b, gn_w, gn_b, scale):
    w = np.asarray(conv_w, dtype=np.float32)
    gwsc = (np.asarray(gn_w, dtype=np.float32)
            * np.asarray(scale, dtype=np.float32).reshape(COUT))
    s = np.where(gwsc >= 0.0, np.float32(1.0), np.float32(-1.0))
    wsgn = (w * s[:, None, None, None]).astype(np.float16)
    wp = np.empty((3, 128, COUT), np.float16)
    ws = np.empty((3, 64, COUT), np.float16)
    for kw in range(3):
        wp[kw, 0:64, :] = wsgn[:, :, 0, kw].T
        wp[kw, 64:128, :] = wsgn[:, :, 1, kw].T
        ws[kw, :, :] = wsgn[:, :, 2, kw].T
    cb = np.asarray(conv_b, dtype=np.float32).reshape(COUT, 1)
    sg = s.reshape(COUT, 1)
    gws = gwsc.reshape(COUT, 1)
    gsa = np.abs(gwsc).reshape(COUT, 1)
    gbs = (np.asarray(gn_b, dtype=np.float32)
           * np.asarray(scale, dtype=np.float32).reshape(COUT)).reshape(COUT, 1)
    bones = np.zeros((COUT, COUT), np.float32)
    for g in range(NG):
        bones[g * GSZ : (g + 1) * GSZ, g * GSZ : (g + 1) * GSZ] = 1.0
    return wp, ws, cb, sg, gws, gsa, gbs, bones


def kernel(x, conv_w, conv_b, gn_w, gn_b, scale):
    x16 = np.ascontiguousarray(np.asarray(x)).astype(np.float16)
    wp, ws, cb, sg, gws, gsa, gbs, bones = _prep_consts(
        conv_w, conv_b, gn_w, gn_b, scale
    )
    nc = _build()
    in_maps = []
    for c in range(N_CORES):
        in_maps.append({
            "xs": x16[c * BPC : (c + 1) * BPC],
            "wp": wp, "ws": ws, "cb": cb, "sg": sg,
            "gws": gws, "gsa": gsa, "gbs": gbs, "bones": bones,
        })
    results = _run_cached(nc, in_maps)
    out = np.concatenate([results[c]["out"] for c in range(N_CORES)], axis=0)
    return np.ascontiguousarray(out.astype(np.float32))


def _run_cached(nc, in_maps):
    """run_bass_kernel_spmd's axon path with the jitted executable cached
    across calls (avoids re-tracing the shard_map wrapper every call)."""
    import jax
    import numpy as _np
    from jax.sharding import Mesh, PartitionSpec
    from jax.experimental.shard_map import shard_map
    from concourse import bass2jax

    if "runner" not in _CACHED:
        bass2jax.install_neuronx_cc_hook()
        partition_name = (
            nc.partition_id_tensor.name if nc.partition_id_tensor else None
        )
        in_names, out_names, out_avals, zero_outs = [], [], [], []
        for alloc in nc.m.functions[0].allocations:
            if not isinstance(alloc, mybir.MemoryLocationSet):
                continue
            name = alloc.memorylocations[0].name
            if alloc.kind == "ExternalInput":
                if name != partition_name:
                    in_names.append(name)
            elif alloc.kind == "ExternalOutput":
                shape = tuple(alloc.tensor_shape)
                dtype = mybir.dt.np(alloc.dtype)
                out_names.append(name)
                out_avals.append(jax.core.ShapedArray(shape, dtype))
                zero_outs.append(_np.zeros(shape, dtype))
        n_params = len(in_names)
        n_outs = len(out_avals)
        all_names = list(in_names) + list(out_names)
        if partition_name is not None:
            all_names.append(partition_name)
        donate = tuple(range(n_params, n_params + n_outs))

        def _body(*args):
            operands = list(args)
            if partition_name is not None:
                operands.append(bass2jax.partition_id_tensor())
            outs = bass2jax._bass_exec_p.bind(
                *operands,
                out_avals=tuple(out_avals),
                in_names=tuple(all_names),
                out_names=tuple(out_names),
                lowering_input_output_aliases=(),
                sim_require_finite=True,
                sim_require_nnan=True,
                nc=nc,
            )
            return tuple(outs)

        devices = jax.devices()[:N_CORES]
        mesh = Mesh(_np.asarray(devices), ("core",))
        in_specs = (PartitionSpec("core"),) * (n_params + n_outs)
        out_specs = (PartitionSpec("core"),) * n_outs
        sharded = jax.jit(
            shard_map(_body, mesh=mesh, in_specs=in_specs,
                      out_specs=out_specs, check_rep=False),
            donate_argnums=donate, keep_unused=True,
        )
        _CACHED["runner"] = (sharded, in_names, out_names, out_avals, zero_outs)

    sharded, in_names, out_names, out_avals, zero_outs = _CACHED["runner"]
    import numpy as _np2
    concat_in = [
        _np2.concatenate([_np2.asarray(in_maps[c][n]) for c in range(N_CORES)], axis=0)
        for n in in_names
    ]
    concat_zeros = [
        _np2.zeros((N_CORES * z.shape[0], *z.shape[1:]), z.dtype) for z in zero_outs
    ]
    out_arrs = sharded(*concat_in, *concat_zeros)
    return [
        {
            name: _np2.asarray(out_arrs[i]).reshape(N_CORES, *out_avals[i].shape)[c]
            for i, name in enumerate(out_names)
        }
        for c in range(N_CORES)
    ]


if __name__ == "__main__":
    rng = np.random.default_rng(0)
    x = rng.standard_normal((B_FULL, CIN, H, W), dtype=np.float32)
    cw = rng.standard_normal((COUT, CIN, 3, 3), dtype=np.float32)
    out = kernel(x, cw, rng.standard_normal(COUT, dtype=np.float32),
                 rng.standard_normal(COUT, dtype=np.float32),
                 rng.standard_normal(COUT, dtype=np.float32),
                 rng.standard_normal((COUT, 1, 1), dtype=np.float32))
    print(out.shape, out.dtype)
